# revision 4
# baseline (speedup 1.0000x reference)
"""Trainium2 Bass kernel for clustered (sorted-group) multi-head attention.

Full inputs in, full output out. Host does the data-dependent token sort
(argmax over sim + stable argsort) and layout packing; the 1024 independent
128-token attention groups are sharded 128-per-core across 8 NeuronCores.
Device computes, per group and head: S = scale*q@k^T, softmax(S), O = P@v,
then Y = O @ W^T (heads accumulated in PSUM). Bias-add and un-sort on host.
"""

import os
import numpy as np

NUM_HEADS = 4
GS = 128          # tokens per category group
HD = 48           # head dim
CDIM = 192        # channels
B = 2
N = 65536
NCORES = 8
NG = (B * N) // GS            # 1024 total groups
GCORE = NG // NCORES          # 128 groups per core
CHUNK = 16                    # groups per DMA chunk
NCHUNK = GCORE // CHUNK

_cache = {}
LAST_RESULT = None


def _build_nc():
    import concourse.bass as bass  # noqa: F401
    import concourse.mybir as mybir
    from concourse import bacc
    from concourse.tile import TileContext

    dt = mybir.dt
    f32, f16 = dt.float32, dt.float16

    nc = bacc.Bacc(None, target_bir_lowering=False)
    qt_e = nc.declare_dram_parameter("qt", [2, GCORE // 2, HD, NUM_HEADS * GS], f16, isOutput=False)
    kt_e = nc.declare_dram_parameter("kt", [2, GCORE // 2, HD, NUM_HEADS * GS], f16, isOutput=False)
    v_e = nc.declare_dram_parameter("v", [GCORE, GS, CDIM], f16, isOutput=False)
    wt_e = nc.declare_dram_parameter("wt", [NUM_HEADS, HD, CDIM], f16, isOutput=False)
    id_e = nc.declare_dram_parameter("ident", [GS, GS], f16, isOutput=False)
    out_e = nc.declare_dram_parameter("out", [GCORE, GS, CDIM], f16, isOutput=True)

    FW = NUM_HEADS * GS  # 512 free elems per group in qt/kt

    with TileContext(nc) as tc:
        with (
            tc.tile_pool(name="consts", bufs=1) as consts,
            tc.tile_pool(name="qk", bufs=2) as qk_pool,
            tc.tile_pool(name="vp", bufs=2) as v_pool,
            tc.tile_pool(name="op", bufs=2) as o_pool,
            tc.tile_pool(name="pp", bufs=2) as p_pool,
            tc.tile_pool(name="st", bufs=3) as st_pool,
            tc.tile_pool(name="ps_s", bufs=2, space="PSUM") as ps_s,
            tc.tile_pool(name="ps_t", bufs=2, space="PSUM") as ps_t,
            tc.tile_pool(name="ps_o", bufs=2, space="PSUM") as ps_o,
            tc.tile_pool(name="ps_y", bufs=2, space="PSUM") as ps_y,
        ):
            wt_t = consts.tile([112, NUM_HEADS, CDIM], f16)
            nc.sync.dma_start(out=wt_t[0:48], in_=wt_e[:, :, :].rearrange("h p f -> p h f"))
            nc.sync.dma_start(out=wt_t[64:112], in_=wt_e[:, :, :].rearrange("h p f -> p h f"))
            id_t = consts.tile([GS, GS], f16)
            nc.sync.dma_start(out=id_t, in_=id_e[:, :])

            for ci in range(NCHUNK):
                gsl = slice(ci * CHUNK, (ci + 1) * CHUNK)
                hsl = slice(ci * (CHUNK // 2), (ci + 1) * (CHUNK // 2))
                q_t = qk_pool.tile([112, CHUNK // 2, FW], f16, tag="q_t")
                k_t = qk_pool.tile([112, CHUNK // 2, FW], f16, tag="k_t")
                nc.sync.dma_start(out=q_t[0:48, :], in_=qt_e[0, hsl].rearrange("c p f -> p c f"))
                nc.sync.dma_start(out=q_t[64:112, :], in_=qt_e[1, hsl].rearrange("c p f -> p c f"))
                nc.sync.dma_start(out=k_t[0:48, :], in_=kt_e[0, hsl].rearrange("c p f -> p c f"))
                nc.sync.dma_start(out=k_t[64:112, :], in_=kt_e[1, hsl].rearrange("c p f -> p c f"))
                v_t = v_pool.tile([GS, CHUNK, CDIM], f16)
                nc.sync.dma_start(out=v_t, in_=v_e[gsl].rearrange("c p f -> p c f"))
                out_t = o_pool.tile([GS, CHUNK, CDIM], f16)

                for gi in range(CHUNK):
                    qb = 64 * (gi % 2)
                    gp = gi // 2
                    s4 = ps_s.tile([GS, NUM_HEADS, GS], f32)
                    for h in range(NUM_HEADS):
                        nc.tensor.matmul(
                            s4[:, h],
                            lhsT=q_t[qb : qb + 48, gp, h * GS : (h + 1) * GS],
                            rhs=k_t[qb : qb + 48, gp, h * GS : (h + 1) * GS],
                            start=True, stop=True,
                        )
                    negm4 = st_pool.tile([GS, NUM_HEADS], f32, tag="negm")
                    nc.vector.tensor_reduce(
                        negm4, s4[:, :, :], axis=mybir.AxisListType.X,
                        op=mybir.AluOpType.max, negate=True,
                    )
                    p4 = p_pool.tile([GS, NUM_HEADS, GS], f16, tag="p4")
                    l4 = st_pool.tile([GS, NUM_HEADS], f32, tag="l4")
                    for h in range(NUM_HEADS):
                        nc.scalar.activation(
                            p4[:, h], s4[:, h],
                            mybir.ActivationFunctionType.Exp,
                            bias=negm4[:, h : h + 1], scale=1.0,
                            accum_out=l4[:, h : h + 1],
                        )
                    r4 = st_pool.tile([GS, NUM_HEADS], f32, tag="r4")
                    nc.vector.reciprocal(r4, l4)
                    pn4 = p_pool.tile([GS, NUM_HEADS, GS], f16, tag="pn4")
                    for h in range(NUM_HEADS):
                        nc.gpsimd.tensor_scalar_mul(pn4[:, h], p4[:, h], r4[:, h : h + 1])
                    pt4 = ps_t.tile([GS, NUM_HEADS, GS], f16)
                    for h in range(NUM_HEADS):
                        nc.tensor.transpose(pt4[:, h], pn4[:, h], id_t)
                    pt4_sb = p_pool.tile([GS, NUM_HEADS, GS], f16, tag="pt4")
                    nc.vector.tensor_copy(pt4_sb[:, :, :], pt4[:, :, :])
                    o4 = ps_o.tile([HD, NUM_HEADS, GS], f32)
                    for h in range(NUM_HEADS):
                        nc.tensor.matmul(
                            o4[:, h],
                            lhsT=v_t[:, gi, h * HD : (h + 1) * HD],
                            rhs=pt4_sb[:, h],
                            start=True, stop=True,
                        )
                    ot4 = p_pool.tile([HD, NUM_HEADS, GS], f16, tag="ot4")
                    nc.vector.tensor_copy(ot4[:, :, :], o4[:, :, :])
                    y = ps_y.tile([GS, CDIM], f32)
                    for h in range(NUM_HEADS):
                        nc.tensor.matmul(
                            y,
                            lhsT=ot4[:, h],
                            rhs=wt_t[0:48, h],
                            start=(h == 0), stop=(h == NUM_HEADS - 1),
                        )
                    nc.scalar.copy(out_t[:, gi], y)

                nc.sync.dma_start(
                    out=out_e[gsl].rearrange("c p f -> p c f"), in_=out_t
                )

    nc.finalize()
    return nc


def kernel(qkv, sim, proj_w, proj_b, logit_scale, H=None, W=None, **_):
    global LAST_RESULT
    from concourse.bass_utils import run_bass_kernel_spmd

    qkv = np.asarray(qkv, dtype=np.float32)
    sim = np.asarray(sim, dtype=np.float32)
    proj_w = np.asarray(proj_w, dtype=np.float32)
    proj_b = np.asarray(proj_b, dtype=np.float32)
    scale = float(np.exp(min(float(np.asarray(logit_scale).reshape(-1)[0]), np.log(100.0))))

    b, n, c3 = qkv.shape
    assert (b, n, c3) == (B, N, 3 * CDIM)

    # --- host: cluster sort (data-dependent reorder = the sharding step) ---
    tk = np.argmax(sim, axis=-1)                          # (b, n)
    idx = np.argsort(tk, axis=-1, kind="stable")          # (b, n)
    srt = np.take_along_axis(qkv, idx[..., None], axis=1) # (b, n, 576)
    grp = srt.reshape(NG, GS, 3 * CDIM)                   # (1024, 128, 576)

    q = grp[:, :, :CDIM].reshape(NG, GS, NUM_HEADS, HD)
    k = grp[:, :, CDIM : 2 * CDIM].reshape(NG, GS, NUM_HEADS, HD)
    qt = np.ascontiguousarray(q.transpose(0, 3, 2, 1))    # (g, d, h, t)
    qt = (qt * scale).astype(np.float16).reshape(NG, HD, NUM_HEADS * GS)
    kt = np.ascontiguousarray(k.transpose(0, 3, 2, 1)).astype(np.float16)
    kt = kt.reshape(NG, HD, NUM_HEADS * GS)
    v = np.ascontiguousarray(grp[:, :, 2 * CDIM :]).astype(np.float16)  # (g, t, c)
    wt = np.ascontiguousarray(proj_w.T.reshape(NUM_HEADS, HD, CDIM)).astype(np.float16)
    ident = np.eye(GS, dtype=np.float16)

    key = "nc"
    if key not in _cache:
        _cache[key] = _build_nc()
    nc = _cache[key]

    in_maps = []
    for i in range(NCORES):
        gs_ = slice(i * GCORE, (i + 1) * GCORE)
        qt_c, kt_c = qt[gs_], kt[gs_]
        in_maps.append({
            "qt": np.ascontiguousarray(np.stack([qt_c[0::2], qt_c[1::2]])),
            "kt": np.ascontiguousarray(np.stack([kt_c[0::2], kt_c[1::2]])),
            "v": v[gs_], "wt": wt, "ident": ident,
        })

    trace = bool(os.environ.get("BASS_TRACE"))
    res = run_bass_kernel_spmd(nc, in_maps, core_ids=list(range(NCORES)), trace=trace)
    LAST_RESULT = res

    out_sorted = np.concatenate(
        [np.asarray(res.results[i]["out"], dtype=np.float32) for i in range(NCORES)],
        axis=0,
    )                                                     # (1024, 128, 192)
    out_sorted = out_sorted.reshape(B, N, CDIM) + proj_b[None, None, :]
    out = np.empty((B, N, CDIM), dtype=np.float32)
    np.put_along_axis(out, idx[..., None], out_sorted, axis=1)
    return out


# revision 9
# speedup vs baseline: 3.9235x; 3.9235x over previous
"""Trainium2 Bass kernel for clustered (sorted-group) multi-head attention.

Full inputs in, full output out. Host does the data-dependent token sort
(argmax over sim + stable argsort) and layout packing; the 1024 independent
128-token attention groups are sharded 128-per-core across 8 NeuronCores.
Device computes, per group and head: S = scale*q@k^T, softmax(S), O = P@v,
then Y = O @ W^T (heads accumulated in PSUM). Bias-add and un-sort on host.
"""

import os
import numpy as np
import ml_dtypes

NUM_HEADS = 4
GS = 128          # tokens per category group
HD = 48           # head dim
CDIM = 192        # channels
B = 2
N = 65536
NCORES = 8
NG = (B * N) // GS            # 1024 total groups
GCORE = NG // NCORES          # 128 groups per core
CHUNK = 16                    # groups per DMA chunk
NCHUNK = GCORE // CHUNK

_cache = {}
LAST_RESULT = None

# build-time feature flags (bisect aids)
F_HEAD_PARITY = os.environ.get("K_HEAD_PARITY", "0") == "1"
F_BF16 = os.environ.get("K_BF16", "1") == "1"
F_EXP_ACCUM = os.environ.get("K_EXP_ACCUM", "0") == "1"
F_OTCOPY_ACT = os.environ.get("K_OTCOPY_ACT", "1") == "1"


def _build_nc():
    import concourse.bass as bass
    import concourse.mybir as mybir
    from concourse import bacc
    from concourse.tile import TileContext

    dt = mybir.dt
    f32, f16 = dt.float32, dt.float16
    p16 = dt.bfloat16 if F_BF16 else dt.float16

    nc = bacc.Bacc(None, target_bir_lowering=False)
    if F_HEAD_PARITY:
        qt_e = nc.declare_dram_parameter("qt", [2, GCORE, HD, 2 * GS], f16, isOutput=False)
        kt_e = nc.declare_dram_parameter("kt", [2, GCORE, HD, 2 * GS], f16, isOutput=False)
    else:
        qt_e = nc.declare_dram_parameter("qt", [2, GCORE // 2, HD, NUM_HEADS * GS], f16, isOutput=False)
        kt_e = nc.declare_dram_parameter("kt", [2, GCORE // 2, HD, NUM_HEADS * GS], f16, isOutput=False)
    v_e = nc.declare_dram_parameter("v", [GCORE, GS, CDIM], p16, isOutput=False)
    wt_e = nc.declare_dram_parameter("wt", [NUM_HEADS, HD, CDIM], p16, isOutput=False)
    id_e = nc.declare_dram_parameter("ident", [GS, GS], p16, isOutput=False)
    out_e = nc.declare_dram_parameter("out", [GCORE, GS, CDIM], f16, isOutput=True)

    FW = NUM_HEADS * GS

    with TileContext(nc) as tc:
        with (
            tc.tile_pool(name="consts", bufs=1) as consts,
            tc.tile_pool(name="qk", bufs=3) as qk_pool,
            tc.tile_pool(name="vp", bufs=3) as v_pool,
            tc.tile_pool(name="op", bufs=2) as o_pool,
            tc.tile_pool(name="pp", bufs=4) as p_pool,
            tc.tile_pool(name="st", bufs=6) as st_pool,
            tc.tile_pool(name="ps_s", bufs=2, space="PSUM") as ps_s,
            tc.tile_pool(name="ps_t", bufs=2, space="PSUM") as ps_t,
            tc.tile_pool(name="ps_o", bufs=2, space="PSUM") as ps_o,
            tc.tile_pool(name="ps_y", bufs=2, space="PSUM") as ps_y,
        ):
            wt_t = consts.tile([112, NUM_HEADS, CDIM], p16)
            nc.sync.dma_start(out=wt_t[0:48], in_=wt_e[:, :, :].rearrange("h p f -> p h f"))
            nc.sync.dma_start(out=wt_t[64:112], in_=wt_e[:, :, :].rearrange("h p f -> p h f"))
            id_t = consts.tile([GS, GS], p16)
            nc.sync.dma_start(out=id_t, in_=id_e[:, :])

            for ci in range(NCHUNK):
                gsl = slice(ci * CHUNK, (ci + 1) * CHUNK)
                if F_HEAD_PARITY:
                    q_t = qk_pool.tile([112, CHUNK, 2 * GS], f16, tag="q_t")
                    k_t = qk_pool.tile([112, CHUNK, 2 * GS], f16, tag="k_t")
                    nc.sync.dma_start(out=q_t[0:48, :], in_=qt_e[0, gsl].rearrange("c p f -> p c f"))
                    nc.sync.dma_start(out=q_t[64:112, :], in_=qt_e[1, gsl].rearrange("c p f -> p c f"))
                    nc.sync.dma_start(out=k_t[0:48, :], in_=kt_e[0, gsl].rearrange("c p f -> p c f"))
                    nc.sync.dma_start(out=k_t[64:112, :], in_=kt_e[1, gsl].rearrange("c p f -> p c f"))
                else:
                    hsl = slice(ci * (CHUNK // 2), (ci + 1) * (CHUNK // 2))
                    q_t = qk_pool.tile([112, CHUNK // 2, FW], f16, tag="q_t")
                    k_t = qk_pool.tile([112, CHUNK // 2, FW], f16, tag="k_t")
                    nc.sync.dma_start(out=q_t[0:48, :], in_=qt_e[0, hsl].rearrange("c p f -> p c f"))
                    nc.sync.dma_start(out=q_t[64:112, :], in_=qt_e[1, hsl].rearrange("c p f -> p c f"))
                    nc.sync.dma_start(out=k_t[0:48, :], in_=kt_e[0, hsl].rearrange("c p f -> p c f"))
                    nc.sync.dma_start(out=k_t[64:112, :], in_=kt_e[1, hsl].rearrange("c p f -> p c f"))
                v_t = v_pool.tile([GS, CHUNK, CDIM], p16)
                nc.sync.dma_start(out=v_t, in_=v_e[gsl].rearrange("c p f -> p c f"))
                out_t = o_pool.tile([GS, CHUNK, CDIM], f16)

                for gi in range(CHUNK):
                    s4 = ps_s.tile([GS, NUM_HEADS, GS], f32)
                    for h in range(NUM_HEADS):
                        if F_HEAD_PARITY:
                            qb = 64 * (h % 2)
                            gp = gi
                            fo = (h // 2) * GS
                        else:
                            qb = 64 * (gi % 2)
                            gp = gi // 2
                            fo = h * GS
                        nc.tensor.matmul(
                            s4[:, h],
                            lhsT=q_t[qb : qb + 48, gp, fo : fo + GS],
                            rhs=k_t[qb : qb + 48, gp, fo : fo + GS],
                            start=True, stop=True,
                        )
                    negm4 = st_pool.tile([GS, NUM_HEADS], f32, tag="negm")
                    nc.vector.tensor_reduce(
                        negm4, s4[:, :, :], axis=mybir.AxisListType.X,
                        op=mybir.AluOpType.max, negate=True,
                    )
                    p4 = p_pool.tile([GS, NUM_HEADS, GS], p16, tag="p4")
                    l4 = st_pool.tile([GS, NUM_HEADS], f32, tag="l4")
                    for h in range(NUM_HEADS):
                        if F_EXP_ACCUM:
                            nc.scalar.activation(
                                p4[:, h], s4[:, h],
                                mybir.ActivationFunctionType.Exp,
                                bias=negm4[:, h : h + 1], scale=1.0,
                                accum_out=l4[:, h : h + 1],
                            )
                        else:
                            nc.scalar.activation(
                                p4[:, h], s4[:, h],
                                mybir.ActivationFunctionType.Exp,
                                bias=negm4[:, h : h + 1], scale=1.0,
                            )
                    if not F_EXP_ACCUM:
                        nc.vector.tensor_reduce(
                            l4, p4[:, :, :], axis=mybir.AxisListType.X,
                            op=mybir.AluOpType.add,
                        )
                    r4 = st_pool.tile([GS, NUM_HEADS], f32, tag="r4")
                    nc.vector.reciprocal(r4, l4)
                    pn4 = p_pool.tile([GS, NUM_HEADS, GS], p16, tag="pn4")
                    pa, ra = bass.broadcast_tensor_aps(p4[:, :, :], r4[:, :, None])
                    nc.vector.tensor_mul(pn4[:, :, :], pa, ra)
                    pt4 = ps_t.tile([GS, NUM_HEADS, GS], p16)
                    for h in range(NUM_HEADS):
                        nc.tensor.transpose(pt4[:, h], pn4[:, h], id_t)
                    pt4_sb = p_pool.tile([GS, NUM_HEADS, GS], p16, tag="pt4")
                    nc.vector.tensor_copy(pt4_sb[:, :, :], pt4[:, :, :])
                    o4 = ps_o.tile([HD, NUM_HEADS, GS], f32)
                    for h in range(NUM_HEADS):
                        nc.tensor.matmul(
                            o4[:, h],
                            lhsT=v_t[:, gi, h * HD : (h + 1) * HD],
                            rhs=pt4_sb[:, h],
                            start=True, stop=True,
                        )
                    ot4 = p_pool.tile([HD, NUM_HEADS, GS], p16, tag="ot4")
                    if F_OTCOPY_ACT:
                        nc.scalar.copy(ot4[:, :, :], o4[:, :, :])
                    else:
                        nc.vector.tensor_copy(ot4[:, :, :], o4[:, :, :])
                    y = ps_y.tile([GS, CDIM], f32)
                    for h in range(NUM_HEADS):
                        nc.tensor.matmul(
                            y,
                            lhsT=ot4[:, h],
                            rhs=wt_t[0:48, h],
                            start=(h == 0), stop=(h == NUM_HEADS - 1),
                        )
                    nc.scalar.copy(out_t[:, gi], y)

                nc.sync.dma_start(
                    out=out_e[gsl].rearrange("c p f -> p c f"), in_=out_t
                )

    nc.finalize()
    return nc


def _pack_qk(qt, kt):
    """qt/kt: (NG, HD, NUM_HEADS, GS) -> dram layouts per F_HEAD_PARITY."""
    if F_HEAD_PARITY:
        # [0] = heads {0,2}, [1] = heads {1,3} -> (2, NG, HD, 2*GS)
        q2 = np.stack([qt[:, :, 0::2], qt[:, :, 1::2]]).reshape(2, NG, HD, 2 * GS)
        k2 = np.stack([kt[:, :, 0::2], kt[:, :, 1::2]]).reshape(2, NG, HD, 2 * GS)
        return np.ascontiguousarray(q2), np.ascontiguousarray(k2)
    return (qt.reshape(NG, HD, NUM_HEADS * GS),
            kt.reshape(NG, HD, NUM_HEADS * GS))


def kernel(qkv, sim, proj_w, proj_b, logit_scale, H=None, W=None, **_):
    global LAST_RESULT
    from concourse.bass_utils import run_bass_kernel_spmd

    qkv = np.asarray(qkv, dtype=np.float32)
    sim = np.asarray(sim, dtype=np.float32)
    proj_w = np.asarray(proj_w, dtype=np.float32)
    proj_b = np.asarray(proj_b, dtype=np.float32)
    scale = float(np.exp(min(float(np.asarray(logit_scale).reshape(-1)[0]), np.log(100.0))))

    b, n, c3 = qkv.shape
    assert (b, n, c3) == (B, N, 3 * CDIM)

    p16 = ml_dtypes.bfloat16 if F_BF16 else np.float16

    # --- host: cluster sort (data-dependent reorder = the sharding step) ---
    tk = np.argmax(sim, axis=-1)                          # (b, n)
    idx = np.argsort(tk, axis=-1, kind="stable")          # (b, n)
    srt = np.take_along_axis(qkv, idx[..., None], axis=1) # (b, n, 576)
    grp = srt.reshape(NG, GS, 3 * CDIM)                   # (1024, 128, 576)

    q = grp[:, :, :CDIM].reshape(NG, GS, NUM_HEADS, HD)
    k = grp[:, :, CDIM : 2 * CDIM].reshape(NG, GS, NUM_HEADS, HD)
    qt = np.ascontiguousarray(q.transpose(0, 3, 2, 1))    # (g, d, h, t)
    qt = (qt * scale).astype(np.float16)                  # (NG, HD, 4, GS)
    kt = np.ascontiguousarray(k.transpose(0, 3, 2, 1)).astype(np.float16)
    qt, kt = _pack_qk(qt, kt)
    v = np.ascontiguousarray(grp[:, :, 2 * CDIM :]).astype(p16)  # (g, t, c)
    wt = np.ascontiguousarray(proj_w.T.reshape(NUM_HEADS, HD, CDIM)).astype(p16)
    ident = np.eye(GS, dtype=p16)

    key = "nc"
    if key not in _cache:
        _cache[key] = _build_nc()
    nc = _cache[key]

    in_maps = []
    for i in range(NCORES):
        gs_ = slice(i * GCORE, (i + 1) * GCORE)
        if F_HEAD_PARITY:
            qs = np.ascontiguousarray(qt[:, gs_])
            ks = np.ascontiguousarray(kt[:, gs_])
        else:
            qc, kc = qt[gs_], kt[gs_]
            qs = np.ascontiguousarray(np.stack([qc[0::2], qc[1::2]]))
            ks = np.ascontiguousarray(np.stack([kc[0::2], kc[1::2]]))
        in_maps.append({"qt": qs, "kt": ks, "v": v[gs_], "wt": wt, "ident": ident})

    trace = bool(os.environ.get("BASS_TRACE"))
    res = run_bass_kernel_spmd(nc, in_maps, core_ids=list(range(NCORES)), trace=trace)
    LAST_RESULT = res

    out_sorted = np.concatenate(
        [np.asarray(res.results[i]["out"], dtype=np.float32) for i in range(NCORES)],
        axis=0,
    )                                                     # (1024, 128, 192)
    out_sorted = out_sorted.reshape(B, N, CDIM) + proj_b[None, None, :]
    out = np.empty((B, N, CDIM), dtype=np.float32)
    np.put_along_axis(out, idx[..., None], out_sorted, axis=1)
    return out
